# revision 1
# baseline (speedup 1.0000x reference)
"""ARMAConv-style GNN message passing kernel for 8 Trainium2 NeuronCores.

Self-contained: host-side index preprocessing + Bass/Tile program builder +
SPMD runner. Algorithm (equivalent to the reference):

    nodes  = node_tensor[:, 0, :]                        # (N, 128)
    out1   = relu(nodes @ (W_pre+W_merge) + (b_pre+b_merge))
    M2     = nodes @ W_merge + (b_merge + b_conv)        # merge + conv bias
    y1     = grouped_conv_nobias(out1)                   # per-node (N, 256)
    z1     = A_hat @ y1        (A_hat = D^-1/2 A D^-1/2, dst-aggregated)
    out2   = relu(z1 + M2)
    y2     = grouped_conv_nobias(out2)
    z2     = A_hat @ y2
    out3   = relu(z2 + M2)
    result[n, c] = (out3[n, 2c] + out3[n, 2c+1]) / 2  (within each group half)

The grouped conv commutes with A_hat (it is per-node linear), so the conv is
applied *before* aggregation and the conv bias is folded into M2.

Sharding: nodes are padded to 50176 = 8 * 49 * 128 and split contiguously
across 8 cores (6272 nodes / 49 blocks of 128 per core). Edges are assigned
to the core owning their destination, sorted by destination, and packed into
128-edge chunks per 128-destination block. Aggregation per chunk is a PE
matmul with host-precomputed weighted one-hot selection matrices streamed
from DRAM (sel[e, d] = (dstloc[e] == d) * norm[src]*norm[dst], split by the
parity of the gathered pair row). Activation tables y1/y2 are exchanged with
segmented AllGathers; per-edge source rows are fetched with gpsimd
dma_gather. Gather descriptor generation on the Q7 cluster (~7ns/edge) is
the serial bottleneck of the aggregation phases; it is minimized by marking
trailing pad slots with index -1 and passing the true per-core edge count in
a register (the Q7 desc-gen kernel trims them), and overlapped with the DMA
transfers by rotating gathers across the four SWDGE queues (ring reclaim of
queue q does not block desc-gen of the next gather on queue q+1). Phase-2
AllGathers are issued a few blocks after their producing segment so the
blocking collective wait on the gpsimd engine overlaps gather desc-gen.
(A prepare_only/trigger_dma descriptor-hoisting variant exists behind
USE_PREP but produces corrupt gathers on hardware despite passing the
simulator; it is disabled.)
"""

import os
import sys

import numpy as np

for _p in ("/opt/trn_rl_repo", "/root/.axon_site/_ro/trn_rl_repo"):
    if os.path.isdir(_p) and _p not in sys.path:
        sys.path.append(_p)

import concourse.bacc as bacc
import concourse.bass as bass
import concourse.mybir as mybir
import concourse.tile as tile
from concourse.bass_utils import run_bass_kernel_spmd
from concourse.masks import make_identity

F32 = mybir.dt.float32
BF16 = mybir.dt.bfloat16
I32 = mybir.dt.int32
I16 = mybir.dt.int16

N_CORES = 8
IC = 128
OC = 128
WIDTH = 2
C2 = OC * WIDTH  # 256


class Cfg:
    def __init__(self, n, nblk_per_core, segs, dt_tab=F32):
        self.n = n                        # real node count
        self.nblk = nblk_per_core         # 128-node blocks per core
        self.percore = 128 * nblk_per_core
        self.npad = N_CORES * self.percore
        self.segs = segs                  # AllGather segments per phase
        assert nblk_per_core % segs == 0
        self.segblk = nblk_per_core // segs
        self.segrows = self.segblk * 128
        self.dt_tab = dt_tab              # dtype of the exchanged y tables
        assert self.npad >= n


def full_cfg(dt_tab=BF16):
    return Cfg(n=50000, nblk_per_core=49, segs=7, dt_tab=dt_tab)


# ---------------------------------------------------------------- host prep

def _y_row_of_node(g, cfg):
    """Map global node id -> row in the (segment-interleaved) y_full table."""
    rank = g // cfg.percore
    r = g % cfg.percore
    s = r // cfg.segrows
    return s * (N_CORES * cfg.segrows) + rank * cfg.segrows + (r % cfg.segrows)


def host_prep(node_tensor, edge_index, W_pre, b_pre, W_merge, b_merge,
              W_conv, b_conv, cfg):
    n = cfg.n
    x = np.ascontiguousarray(np.asarray(node_tensor, np.float32).reshape(n, IC))
    ei = np.asarray(edge_index)
    src = ei[0].astype(np.int64)
    dst = ei[1].astype(np.int64)
    E = src.shape[0]

    deg = np.bincount(dst, minlength=n).astype(np.float32)
    norm = np.where(deg > 0, 1.0 / np.sqrt(np.maximum(deg, 1.0)), 0.0).astype(
        np.float32)
    w = norm[src] * norm[dst]

    order = np.argsort(dst, kind="stable")
    src_s, dst_s, w_s = src[order], dst[order], w[order]

    core_s = dst_s // cfg.percore
    blk_s = (dst_s % cfg.percore) // 128
    grp = core_s * cfg.nblk + blk_s  # non-decreasing in the sorted stream
    counts = np.bincount(grp, minlength=N_CORES * cfg.nblk)
    nk = np.maximum(
        1, -(-counts.reshape(N_CORES, cfg.nblk).max(axis=0) // 128)).astype(
        np.int64)  # chunks per block (same for all cores)
    tot = int(nk.sum())
    koff = np.zeros(cfg.nblk + 1, np.int64)
    koff[1:] = np.cumsum(nk)

    ends = np.cumsum(counts)
    starts = ends - counts
    pos = np.arange(E) - starts[grp]          # position within (core, block)
    col = koff[blk_s] + pos // 128            # chunk column in metadata
    row = pos % 128                           # partition (edge slot in chunk)

    yrow = _y_row_of_node(src_s, cfg)
    parity = (yrow % 2).astype(np.int64)
    # pair index for the gather; -1 marks (trailing) pad slots for desc skip
    pairm = np.full((N_CORES, 128, tot), -1, np.int32)
    pairm[core_s, row, col] = (yrow // 2).astype(np.int32)
    # per-(core, block) true edge counts (for num_idxs_reg)
    cnts = counts.reshape(N_CORES, cfg.nblk).astype(np.int32)
    for c in range(N_CORES):
        for b in range(cfg.nblk):
            if cnts[c, b] == 0:          # ensure >= 1 valid index per call
                pairm[c, 0, koff[b]] = 0
                cnts[c, b] = 1
    # weighted one-hot selection matrices, one [128, 256] slab per chunk:
    # cols [k*256 + par*128 + dstloc]; zero at pad slots.
    selm = np.zeros((N_CORES, 128, tot * 256), np.float32)
    selm[core_s, row, col * 256 + parity * 128 + (dst_s % 128)] = w_s
    # int16-pack pair indices per chunk: [16-wrap, replicated to 128 parts]
    srcp = np.zeros((N_CORES, 128, tot * 8), np.int16)
    for c in range(N_CORES):
        flat = pairm[c].T.reshape(tot * 128)      # edge order e = k*128 + p
        a16 = flat.reshape(tot * 8, 16).T          # [16, tot*8]
        srcp[c] = np.tile(a16, (8, 1)).astype(np.int16)

    np_mm = np.float32
    if cfg.dt_tab != F32:
        import ml_dtypes
        np_mm = ml_dtypes.bfloat16

    xpad = np.zeros((cfg.npad, IC), np.float32)
    xpad[:n] = x
    xT = [np.ascontiguousarray(
        xpad[c * cfg.percore:(c + 1) * cfg.percore].T).astype(np_mm)
        for c in range(N_CORES)]

    W_pre = np.asarray(W_pre, np.float32)
    W_merge = np.asarray(W_merge, np.float32)
    W_conv = np.asarray(W_conv, np.float32)
    b_pre = np.asarray(b_pre, np.float32)
    b_merge = np.asarray(b_merge, np.float32)
    b_conv = np.asarray(b_conv, np.float32)

    wsum = np.ascontiguousarray(W_pre + W_merge).astype(np_mm)    # (128, 256)
    wmrg = np.ascontiguousarray(W_merge).astype(np_mm)            # (128, 256)
    wct = np.ascontiguousarray(
        np.concatenate([W_conv[0].T, W_conv[1].T], axis=1)).astype(
        np_mm)                                                    # (128, 256)
    b1t = np.tile((b_pre + b_merge)[None, :], (128, 1)).astype(np.float32)
    b2t = np.tile((b_merge + b_conv)[None, :], (128, 1)).astype(np.float32)

    shared = dict(wsum=wsum, wmrg=wmrg, wct=wct, b1t=b1t, b2t=b2t)
    per_core = [dict(xT=xT[c], srcp=np.ascontiguousarray(srcp[c]),
                     selm=np.ascontiguousarray(selm[c]).astype(np_mm),
                     cnts=np.ascontiguousarray(cnts[c][None, :]),
                     **shared)
                for c in range(N_CORES)]
    return per_core, tuple(int(v) for v in nk)


# ------------------------------------------------------------------ builder

USE_PREP = False


def build_program(cfg, nk, use_prep=None):
    if use_prep is None:
        use_prep = USE_PREP
    tot = int(sum(nk))
    nkmax = int(max(nk))
    koff = [0]
    for v in nk:
        koff.append(koff[-1] + v)
    nc = bacc.Bacc("TRN2", target_bir_lowering=False, debug=False,
                   num_devices=N_CORES, num_swdge_queues=4)
    dt = cfg.dt_tab
    dtm = dt  # dtype of the PE matmul path (bf16 when dt_tab is bf16)

    xT_d = nc.dram_tensor("xT", [128, cfg.percore], dtm, kind="ExternalInput")
    srcp_d = nc.dram_tensor("srcp", [128, tot * 8], I16, kind="ExternalInput")
    selm_d = nc.dram_tensor("selm", [128, tot * 256], dtm,
                            kind="ExternalInput")
    cnts_d = nc.dram_tensor("cnts", [1, cfg.nblk], I32, kind="ExternalInput")
    wsum_d = nc.dram_tensor("wsum", [128, C2], dtm, kind="ExternalInput")
    wmrg_d = nc.dram_tensor("wmrg", [128, C2], dtm, kind="ExternalInput")
    wct_d = nc.dram_tensor("wct", [128, C2], dtm, kind="ExternalInput")
    b1_d = nc.dram_tensor("b1t", [128, C2], F32, kind="ExternalInput")
    b2_d = nc.dram_tensor("b2t", [128, C2], F32, kind="ExternalInput")
    out_d = nc.dram_tensor("out", [cfg.percore, 128], F32,
                           kind="ExternalOutput")

    rg = [list(range(N_CORES))]
    add = mybir.AluOpType.add
    relu = mybir.ActivationFunctionType.Relu

    # prep-ahead depth, bounded by the SWDGE descriptor carveout (1024 slots
    # per queue; each per-block gather reserves nk[b]*8+1 m2s slots).
    G = 0
    budget = 840
    acc = 0
    for b in range(cfg.nblk):
        acc += nk[b] * 8 + 1
        if acc > budget:
            break
        G += 1
    G = max(3, min(G, 14))
    if not use_prep:
        G = 8

    dma_sems = [nc.alloc_semaphore(f"swdge_dma{q}") for q in (0, 1)]
    prep_sems = [nc.alloc_semaphore(f"swdge_prep{q}") for q in (0, 1)]
    cnt_regs = [nc.alloc_register(mybir.EngineType.Pool, f"gcnt{b}")
                for b in range(cfg.nblk)]

    with tile.TileContext(nc) as tc:
        with (
            tc.tile_pool(name="const", bufs=1) as cp,
            tc.tile_pool(name="work", bufs=3) as wp,
            tc.tile_pool(name="psum", bufs=2, space="PSUM") as pp,
            tc.tile_pool(name="dram", bufs=1, space="DRAM") as dp,
        ):
            def cload(dram, shape, dtype, name):
                t = cp.tile(shape, dtype, name=name, tag=name)
                nc.sync.dma_start(out=t[:, :], in_=dram[:, :])
                return t

            xT_sb = cload(xT_d, [128, cfg.percore], dtm, "xT_sb")
            srcp_sb = cload(srcp_d, [128, tot * 8], I16, "srcp_sb")
            cnts_sb = cload(cnts_d, [1, cfg.nblk], I32, "cnts_sb")
            wsum_sb = cload(wsum_d, [128, C2], dtm, "wsum_sb")
            wmrg_sb = cload(wmrg_d, [128, C2], dtm, "wmrg_sb")
            wct_sb = cload(wct_d, [128, C2], dtm, "wct_sb")
            b1_sb = cload(b1_d, [128, C2], F32, "b1_sb")
            b2_sb = cload(b2_d, [128, C2], F32, "b2_sb")

            for b in range(cfg.nblk):
                nc.gpsimd.reg_load(cnt_regs[b], cnts_sb[0:1, b:b + 1])

            ident = cp.tile([128, 128], dtm, name="ident", tag="ident")
            make_identity(nc, ident[:, :])

            y_own = [[dp.tile([cfg.segrows, C2], dt, name=f"y{p}own{s}",
                              tag=f"y{p}own{s}")
                      for s in range(cfg.segs)] for p in (0, 1)]
            yf_space = "Shared" if cfg.segs == 1 else "Local"
            y_full = [dp.tile([cfg.npad, C2], dt, addr_space=yf_space,
                              name=f"y{p}full", tag=f"y{p}full")
                      for p in (0, 1)]

            # scrub the gather landing buffers once: pad slots are never
            # written by the DMA (indices -1 are skipped), and sel weight 0
            # only kills *finite* garbage.
            for _ in range(G):
                gt0 = wp.tile([128, nkmax, 2 * C2], dt, tag="gath", bufs=G)
                nc.gpsimd.memset(gt0[:, :, :], 0.0)

            # ---- gather prep/trigger machinery -------------------------
            prep_state = {"j": 0, "fired": [0, 0], "emitted": [0, 0], "burst": [0, 0]}

            def emit_prep():
                """Generate gather descriptors for the next logical block,
                up to G blocks ahead of consumption (prepare_only)."""
                if not use_prep:
                    return
                j = prep_state["j"]
                if j >= 2 * cfg.nblk:
                    return
                prep_state["j"] += 1
                q = j // cfg.nblk            # 0: phase-2 (y1), 1: phase-3 (y2)
                b = j % cfg.nblk
                nkb, k0 = nk[b], koff[b]
                gt = wp.tile([128, nkmax, 2 * C2], dt, tag="gath", bufs=G)
                gt_tiles[j] = gt
                pairs = y_full[q].rearrange("(n two) c -> n (two c)", two=2)
                nc.gpsimd.dma_gather(
                    out_ap=gt[:, :nkb, :], in_ap=pairs,
                    idxs_ap=srcp_sb[:, k0 * 8:(k0 + nkb) * 8],
                    num_idxs=nkb * 128, num_idxs_reg=cnt_regs[b],
                    elem_size=2 * C2, single_packet=False,
                    prepare_only=True, sem=dma_sems[q], queue_num=q)
                prep_state["emitted"][q] += 1

            def fire_next(q, i):
                """Fire pending preps on q (Tile-managed count=None), then
                gate the consuming matmuls of the i-th block on this queue
                on the actual DMA completion. Completions within a queue
                land in firing order except inside the phase-start burst,
                where the conservative threshold covers the whole burst."""
                if prep_state["emitted"][q] > prep_state["fired"][q]:
                    nc.gpsimd.trigger_dma(count=None, queue_num=q)
                    prep_state["fired"][q] = prep_state["emitted"][q]
                    if prep_state["burst"][q] == 0:
                        prep_state["burst"][q] = prep_state["fired"][q]
                burst = prep_state["burst"][q]
                thr = burst if i < burst else i + 1
                nc.tensor.wait_ge(dma_sems[q], 16 * thr)

            def direct_gather(b, j):
                """Non-prepared gather, issued at consume time (fallback)."""
                nkb, k0 = nk[b], koff[b]
                gt = wp.tile([128, nkmax, 2 * C2], dt, tag="gath", bufs=G)
                gt_tiles[j] = gt
                pairs = y_full[j // cfg.nblk].rearrange(
                    "(n two) c -> n (two c)", two=2)
                nc.gpsimd.dma_gather(
                    out_ap=gt[:, :nkb, :], in_ap=pairs,
                    idxs_ap=srcp_sb[:, k0 * 8:(k0 + nkb) * 8],
                    num_idxs=nkb * 128, num_idxs_reg=cnt_regs[b],
                    elem_size=2 * C2, single_packet=False,
                    queue_num=j % 4)

            gt_tiles = {}

            def conv_rows(src_sb):
                """src [128n, 256] f32 -> grouped conv rows [128n, 256] dt."""
                yps = pp.tile([128, C2], F32, tag="mmB")
                for g in (0, 1):
                    tp = pp.tile([128, 128], dtm, tag="tps")
                    nc.tensor.transpose(out=tp[:, :],
                                        in_=src_sb[:, g * 128:(g + 1) * 128],
                                        identity=ident[:, :])
                    tsb = wp.tile([128, 128], dtm, tag="tsb")
                    if g == 0:
                        nc.vector.tensor_copy(out=tsb[:, :], in_=tp[:, :])
                    else:
                        nc.scalar.copy(out=tsb[:, :], in_=tp[:, :])
                    nc.tensor.matmul(out=yps[:, g * 128:(g + 1) * 128],
                                     lhsT=tsb[:, :],
                                     rhs=wct_sb[:, g * 128:(g + 1) * 128],
                                     start=True, stop=True)
                ysb = wp.tile([128, C2], dt, tag="ysb")
                nc.scalar.copy(out=ysb[:, :], in_=yps[:, :])
                return ysb

            def spmm_block(b, j):
                """Aggregate one 128-dst block from the prepared gather j:
                fire the DMA, stream the sel slab, accumulate in PSUM."""
                nkb, k0 = nk[b], koff[b]
                sel = wp.tile([128, nkmax * 256], dt, tag="sel", bufs=6)
                nc.sync.dma_start(
                    out=sel[:, :nkb * 256],
                    in_=selm_d[:, k0 * 256:(k0 + nkb) * 256])
                if use_prep:
                    fire_next(j // cfg.nblk, j % cfg.nblk)
                else:
                    direct_gather(b, j)
                gt = gt_tiles.pop(j)
                zps = pp.tile([128, C2], F32, tag="mmA")
                # merge term x_b @ W_merge opens the accumulation group; it
                # only reads resident tiles, so the PE runs it while the
                # gather DMA is still landing.
                nc.tensor.matmul(out=zps[:, :],
                                 lhsT=xT_sb[:, b * 128:(b + 1) * 128],
                                 rhs=wmrg_sb[:, :], start=True, stop=False)
                mi = 0
                for k in range(nkb):
                    for par, off in ((0, 0), (1, C2)):
                        nc.tensor.matmul(
                            out=zps[:, :],
                            lhsT=sel[:, (2 * k + par) * 128:
                                     (2 * k + par + 1) * 128],
                            rhs=gt[:, k, off:off + C2],
                            start=False, stop=(mi == 2 * nkb - 1))
                        mi += 1
                return zps

            # ---------------- phase 1: dense init + y1 rows
            for b in range(cfg.nblk):
                xb = xT_sb[:, b * 128:(b + 1) * 128]
                ps1 = pp.tile([128, C2], F32, tag="mmA")
                nc.tensor.matmul(out=ps1[:, :], lhsT=xb, rhs=wsum_sb[:, :],
                                 start=True, stop=True)
                o1 = wp.tile([128, C2], F32, tag="o1")
                nc.vector.tensor_tensor(out=o1[:, :], in0=ps1[:, :],
                                        in1=b1_sb[:, :], op=add)
                o1r = wp.tile([128, C2], dtm, tag="o1r")
                nc.scalar.activation(out=o1r[:, :], in_=o1[:, :], func=relu)
                ysb = conv_rows(o1r)
                s, jseg = b // cfg.segblk, b % cfg.segblk
                nc.sync.dma_start(
                    out=y_own[0][s][jseg * 128:(jseg + 1) * 128, :],
                    in_=ysb[:, :])
                if jseg == cfg.segblk - 1:
                    nc.gpsimd.collective_compute(
                        "AllGather", mybir.AluOpType.bypass, replica_groups=rg,
                        ins=[y_own[0][s][:, :]],
                        outs=[y_full[0][s * N_CORES * cfg.segrows:
                                        (s + 1) * N_CORES * cfg.segrows, :]])
                # hoist phase-2 descriptor generation into phase 1 (after the
                # AG so the deferred table dep is carried by some pending prep)
                if b >= cfg.nblk - G:
                    emit_prep()

            # ---------------- phase 2: z1 -> out2 -> y2 rows
            # AG for segment s is issued AG_DELAY blocks after its last
            # producer block, so the CC-busy wait overlaps gather desc-gen
            # instead of stalling it at the gpsimd engine head.
            AG_DELAY = 5

            def emit_ag2(s):
                nc.gpsimd.collective_compute(
                    "AllGather", mybir.AluOpType.bypass, replica_groups=rg,
                    ins=[y_own[1][s][:, :]],
                    outs=[y_full[1][s * N_CORES * cfg.segrows:
                                    (s + 1) * N_CORES * cfg.segrows, :]])

            for b in range(cfg.nblk):
                zps = spmm_block(b, b)
                o2 = wp.tile([128, C2], F32, tag="o1")
                nc.vector.tensor_tensor(out=o2[:, :], in0=zps[:, :],
                                        in1=b2_sb[:, :], op=add)
                o2r = wp.tile([128, C2], dtm, tag="o1r")
                nc.scalar.activation(out=o2r[:, :], in_=o2[:, :], func=relu)
                ysb = conv_rows(o2r)
                s, jseg = b // cfg.segblk, b % cfg.segblk
                nc.sync.dma_start(
                    out=y_own[1][s][jseg * 128:(jseg + 1) * 128, :],
                    in_=ysb[:, :])
                if (b >= cfg.segblk - 1 + AG_DELAY
                        and (b - AG_DELAY) % cfg.segblk == cfg.segblk - 1):
                    emit_ag2((b - AG_DELAY) // cfg.segblk)
                emit_prep()
            for s in range((cfg.nblk - AG_DELAY) // cfg.segblk, cfg.segs):
                emit_ag2(s)

            # ---------------- phase 3: z2 -> out3 -> pooled output rows
            for b in range(cfg.nblk):
                zps = spmm_block(b, cfg.nblk + b)
                o3 = wp.tile([128, C2], F32, tag="o1")
                nc.vector.tensor_tensor(out=o3[:, :], in0=zps[:, :],
                                        in1=b2_sb[:, :], op=add)
                o3r = wp.tile([128, C2], F32, tag="o3r")
                nc.scalar.activation(out=o3r[:, :], in_=o3[:, :], func=relu)
                res = wp.tile([128, 128], F32, tag="res")
                for g in (0, 1):
                    ev = o3r[:, g * 128:(g + 1) * 128:2]
                    od = o3r[:, g * 128 + 1:(g + 1) * 128:2]
                    pm = wp.tile([128, 64], F32, tag="pm")
                    nc.vector.tensor_tensor(out=pm[:, :], in0=ev, in1=od,
                                            op=add)
                    nc.scalar.activation(
                        out=res[:, g * 64:(g + 1) * 64], in_=pm[:, :],
                        func=mybir.ActivationFunctionType.Copy, scale=0.5)
                nc.sync.dma_start(out=out_d[b * 128:(b + 1) * 128, :],
                                  in_=res[:, :])
                emit_prep()

    nc.finalize()
    return nc


# ------------------------------------------------------------------ runner

_PROG_CACHE = {}


def get_program(cfg, nk):
    key = (cfg.n, cfg.nblk, cfg.segs, str(cfg.dt_tab), nk, USE_PREP)
    if key not in _PROG_CACHE:
        _PROG_CACHE[key] = build_program(cfg, nk)
    return _PROG_CACHE[key]


def kernel(node_tensor, edge_index, W_pre, b_pre, W_merge, b_merge,
           W_conv, b_conv, _cfg=None, **_run_kwargs):
    cfg = _cfg or full_cfg()
    per_core, nk = host_prep(node_tensor, edge_index, W_pre, b_pre, W_merge,
                             b_merge, W_conv, b_conv, cfg)
    nc = get_program(cfg, nk)
    res = run_bass_kernel_spmd(nc, per_core, core_ids=list(range(N_CORES)),
                               **_run_kwargs)
    outs = np.concatenate([res.results[c]["out"] for c in range(N_CORES)],
                          axis=0)
    full = outs[:cfg.n].astype(np.float32)[:, :, None]
    if _run_kwargs:
        return full, res
    return full



# revision 3
# speedup vs baseline: 1.5545x; 1.5545x over previous
"""ARMAConv-style GNN message passing kernel for 8 Trainium2 NeuronCores.

Self-contained: host-side index preprocessing + Bass/Tile program builder +
SPMD runner. Algorithm (equivalent to the reference):

    nodes  = node_tensor[:, 0, :]                        # (N, 128)
    out1   = relu(nodes @ (W_pre+W_merge) + (b_pre+b_merge))
    M2     = nodes @ W_merge + (b_merge + b_conv)        # merge + conv bias
    y1     = grouped_conv_nobias(out1)                   # per-node (N, 256)
    z1     = A_hat @ y1        (A_hat = D^-1/2 A D^-1/2, dst-aggregated)
    out2   = relu(z1 + M2)
    y2     = grouped_conv_nobias(out2)
    z2     = A_hat @ y2
    out3   = relu(z2 + M2)
    result[n, c] = (out3[n, 2c] + out3[n, 2c+1]) / 2  (within each group half)

The grouped conv commutes with A_hat (it is per-node linear), so the conv is
applied *before* aggregation and the conv bias is folded into M2.

Sharding: nodes are padded to 50176 = 8 * 49 * 128 and split contiguously
across 8 cores (6272 nodes / 49 blocks of 128 per core). Edges are assigned
to the core owning their destination, sorted by destination, and packed into
128-edge chunks per 128-destination block. Aggregation per chunk is a PE
matmul with weighted one-hot selection matrices (sel[e, d] =
(dstloc[e] == d) * norm[src]*norm[dst], split by the parity of the gathered
pair row) kept RESIDENT in SBUF in fp8e4m3. The exchanged activation tables
y1/y2 are also fp8e4m3 (quantization rel-l2 ~2e-3, well under the 2e-2
gate); they live in a SHARED-space DRAM scratchpad so the segmented
AllGathers only write each core's own slice instead of ring-replicating.
Per-edge source rows are fetched with gpsimd dma_gather in 2-row pairs
(pair index fits the int16 index format; sel parity picks the row), rotated
across the four SWDGE queues so ring reclaim overlaps descriptor generation.
"""

import os
import sys

import numpy as np

for _p in ("/opt/trn_rl_repo", "/root/.axon_site/_ro/trn_rl_repo"):
    if os.path.isdir(_p) and _p not in sys.path:
        sys.path.append(_p)

import concourse.bacc as bacc
import concourse.bass as bass
import concourse.mybir as mybir
import concourse.tile as tile
from concourse.bass_utils import run_bass_kernel_spmd
from concourse.masks import make_identity

F32 = mybir.dt.float32
BF16 = mybir.dt.bfloat16
F8 = mybir.dt.float8e4
I32 = mybir.dt.int32
I16 = mybir.dt.int16

N_CORES = 8
IC = 128
OC = 128
WIDTH = 2
C2 = OC * WIDTH  # 256


class Cfg:
    def __init__(self, n, nblk_per_core, segs, dt_tab=F8):
        self.n = n                        # real node count
        self.nblk = nblk_per_core         # 128-node blocks per core
        self.percore = 128 * nblk_per_core
        self.npad = N_CORES * self.percore
        self.segs = segs                  # AllGather segments per phase
        assert nblk_per_core % segs == 0
        self.segblk = nblk_per_core // segs
        self.segrows = self.segblk * 128
        self.dt_tab = dt_tab              # dtype of the exchanged y tables
        assert self.npad >= n


def full_cfg(dt_tab=F8):
    # segs=1: the sim enforces single-writer on Shared DRAM tensors, so the
    # whole per-phase table is exchanged with ONE Shared-output AllGather
    # (own-slice write + barrier) instead of a Local-output ring chain.
    return Cfg(n=50000, nblk_per_core=49, segs=1, dt_tab=dt_tab)


# ---------------------------------------------------------------- host prep

def _y_row_of_node(g, cfg):
    """Map global node id -> row in the (segment-interleaved) y_full table."""
    rank = g // cfg.percore
    r = g % cfg.percore
    s = r // cfg.segrows
    return s * (N_CORES * cfg.segrows) + rank * cfg.segrows + (r % cfg.segrows)


def host_prep(node_tensor, edge_index, W_pre, b_pre, W_merge, b_merge,
              W_conv, b_conv, cfg):
    n = cfg.n
    x = np.ascontiguousarray(np.asarray(node_tensor, np.float32).reshape(n, IC))
    ei = np.asarray(edge_index)
    src = ei[0].astype(np.int64)
    dst = ei[1].astype(np.int64)
    E = src.shape[0]

    deg = np.bincount(dst, minlength=n).astype(np.float32)
    norm = np.where(deg > 0, 1.0 / np.sqrt(np.maximum(deg, 1.0)), 0.0).astype(
        np.float32)
    w = norm[src] * norm[dst]

    order = np.argsort(dst, kind="stable")
    src_s, dst_s, w_s = src[order], dst[order], w[order]

    core_s = dst_s // cfg.percore
    blk_s = (dst_s % cfg.percore) // 128
    grp = core_s * cfg.nblk + blk_s  # non-decreasing in the sorted stream
    counts = np.bincount(grp, minlength=N_CORES * cfg.nblk)
    nk = np.maximum(
        1, -(-counts.reshape(N_CORES, cfg.nblk).max(axis=0) // 128)).astype(
        np.int64)  # chunks per block (same for all cores)
    tot = int(nk.sum())
    koff = np.zeros(cfg.nblk + 1, np.int64)
    koff[1:] = np.cumsum(nk)

    ends = np.cumsum(counts)
    starts = ends - counts
    pos = np.arange(E) - starts[grp]          # position within (core, block)
    col = koff[blk_s] + pos // 128            # chunk column in metadata
    row = pos % 128                           # partition (edge slot in chunk)

    yrow = _y_row_of_node(src_s, cfg)
    parity = (yrow % 2).astype(np.int64)
    # pair index for the gather; -1 marks (trailing) pad slots for desc skip
    pairm = np.full((N_CORES, 128, tot), -1, np.int32)
    pairm[core_s, row, col] = (yrow // 2).astype(np.int32)
    # per-(core, block) true edge counts (for num_idxs_reg)
    cnts = counts.reshape(N_CORES, cfg.nblk).astype(np.int32)
    for c in range(N_CORES):
        for b in range(cfg.nblk):
            if cnts[c, b] == 0:          # ensure >= 1 valid index per call
                pairm[c, 0, koff[b]] = 0
                cnts[c, b] = 1
    # weighted one-hot selection matrices, one [128, 256] slab per chunk:
    # cols [k*256 + par*128 + dstloc]; zero at pad slots.
    selm = np.zeros((N_CORES, 128, tot * 256), np.float32)
    selm[core_s, row, col * 256 + parity * 128 + (dst_s % 128)] = w_s
    # int16-pack pair indices per chunk: [16-wrap, replicated to 128 parts]
    srcp = np.zeros((N_CORES, 128, tot * 8), np.int16)
    for c in range(N_CORES):
        flat = pairm[c].T.reshape(tot * 128)      # edge order e = k*128 + p
        a16 = flat.reshape(tot * 8, 16).T          # [16, tot*8]
        srcp[c] = np.tile(a16, (8, 1)).astype(np.int16)

    np_mm = np.dtype(mybir.dt.np(BF16))
    np_f8 = np.dtype(mybir.dt.np(cfg.dt_tab))

    xpad = np.zeros((cfg.npad, IC), np.float32)
    xpad[:n] = x
    xT = [np.ascontiguousarray(
        xpad[c * cfg.percore:(c + 1) * cfg.percore].T).astype(np_mm)
        for c in range(N_CORES)]

    W_pre = np.asarray(W_pre, np.float32)
    W_merge = np.asarray(W_merge, np.float32)
    W_conv = np.asarray(W_conv, np.float32)
    b_pre = np.asarray(b_pre, np.float32)
    b_merge = np.asarray(b_merge, np.float32)
    b_conv = np.asarray(b_conv, np.float32)

    wsum = np.ascontiguousarray(W_pre + W_merge).astype(np_mm)    # (128, 256)
    wmrg = np.ascontiguousarray(W_merge).astype(np_mm)            # (128, 256)
    wct = np.ascontiguousarray(
        np.concatenate([W_conv[0].T, W_conv[1].T], axis=1)).astype(
        np_mm)                                                    # (128, 256)
    b1t = np.tile((b_pre + b_merge)[None, :], (128, 1)).astype(np.float32)
    b2t = np.tile((b_merge + b_conv)[None, :], (128, 1)).astype(np.float32)

    shared = dict(wsum=wsum, wmrg=wmrg, wct=wct, b1t=b1t, b2t=b2t)
    per_core = [dict(xT=xT[c], srcp=np.ascontiguousarray(srcp[c]),
                     selm=np.ascontiguousarray(selm[c]).astype(np_f8),
                     cnts=np.ascontiguousarray(cnts[c][None, :]),
                     **shared)
                for c in range(N_CORES)]
    return per_core, tuple(int(v) for v in nk)


# ------------------------------------------------------------------ builder

def build_program(cfg, nk):
    tot = int(sum(nk))
    nkmax = int(max(nk))
    koff = [0]
    for v in nk:
        koff.append(koff[-1] + v)
    nc = bacc.Bacc("TRN2", target_bir_lowering=False, debug=False,
                   num_devices=N_CORES, num_swdge_queues=4)
    dt = cfg.dt_tab   # y tables / sel / gather payload dtype (fp8)
    dtm = BF16        # dense matmul dtype

    xT_d = nc.dram_tensor("xT", [128, cfg.percore], dtm, kind="ExternalInput")
    srcp_d = nc.dram_tensor("srcp", [128, tot * 8], I16, kind="ExternalInput")
    selm_d = nc.dram_tensor("selm", [128, tot * 256], dt,
                            kind="ExternalInput")
    cnts_d = nc.dram_tensor("cnts", [1, cfg.nblk], I32, kind="ExternalInput")
    wsum_d = nc.dram_tensor("wsum", [128, C2], dtm, kind="ExternalInput")
    wmrg_d = nc.dram_tensor("wmrg", [128, C2], dtm, kind="ExternalInput")
    wct_d = nc.dram_tensor("wct", [128, C2], dtm, kind="ExternalInput")
    b1_d = nc.dram_tensor("b1t", [128, C2], F32, kind="ExternalInput")
    b2_d = nc.dram_tensor("b2t", [128, C2], F32, kind="ExternalInput")
    out_d = nc.dram_tensor("out", [cfg.percore, 128], F32,
                           kind="ExternalOutput")

    rg = [list(range(N_CORES))]
    add = mybir.AluOpType.add
    relu = mybir.ActivationFunctionType.Relu

    G = 8  # gather landing buffers in flight

    with tile.TileContext(nc) as tc:
        with (
            tc.tile_pool(name="const", bufs=1) as cp,
            tc.tile_pool(name="work", bufs=3) as wp,
            tc.tile_pool(name="psum", bufs=2, space="PSUM") as pp,
            tc.tile_pool(name="dram", bufs=1, space="DRAM") as dp,
        ):
            def cload(dram, shape, dtype, name):
                t = cp.tile(shape, dtype, name=name, tag=name)
                nc.sync.dma_start(out=t[:, :], in_=dram[:, :])
                return t

            xT_sb = cload(xT_d, [128, cfg.percore], dtm, "xT_sb")
            srcp_sb = cload(srcp_d, [128, tot * 8], I16, "srcp_sb")
            selm_sb = cload(selm_d, [128, tot * 256], dt, "selm_sb")
            cnts_sb = cload(cnts_d, [1, cfg.nblk], I32, "cnts_sb")
            wsum_sb = cload(wsum_d, [128, C2], dtm, "wsum_sb")
            wmrg_sb = cload(wmrg_d, [128, C2], dtm, "wmrg_sb")
            wct_sb = cload(wct_d, [128, C2], dtm, "wct_sb")
            b1_sb = cload(b1_d, [128, C2], F32, "b1_sb")
            b2_sb = cload(b2_d, [128, C2], F32, "b2_sb")

            cnt_regs = [nc.alloc_register(mybir.EngineType.Pool, f"gcnt{b}")
                        for b in range(cfg.nblk)]
            for b in range(cfg.nblk):
                nc.gpsimd.reg_load(cnt_regs[b], cnts_sb[0:1, b:b + 1])

            ident = cp.tile([128, 128], dtm, name="ident", tag="ident")
            make_identity(nc, ident[:, :])

            y_own = [[dp.tile([cfg.segrows, C2], dt, name=f"y{p}own{s}",
                              tag=f"y{p}own{s}")
                      for s in range(cfg.segs)] for p in (0, 1)]
            y_full = [dp.tile([cfg.npad, C2], dt, addr_space="Shared",
                              name=f"y{p}full", tag=f"y{p}full")
                      for p in (0, 1)]

            # scrub the gather landing buffers once: pad slots are never
            # written by the DMA (indices -1 are skipped), and sel weight 0
            # only kills *finite* garbage.
            for _ in range(G):
                gt0 = wp.tile([128, nkmax, 2 * C2], dt, tag="gath", bufs=G)
                nc.gpsimd.memset(gt0[:, :, :], 0.0)

            gt_tiles = {}

            def direct_gather(b, j):
                """Gather the pair rows for block b's edges from the y table
                (phase j // nblk) into a fresh landing tile."""
                nkb, k0 = nk[b], koff[b]
                gt = wp.tile([128, nkmax, 2 * C2], dt, tag="gath", bufs=G)
                gt_tiles[j] = gt
                pairs = y_full[j // cfg.nblk].rearrange(
                    "(n two) c -> n (two c)", two=2)
                nc.gpsimd.dma_gather(
                    out_ap=gt[:, :nkb, :], in_ap=pairs,
                    idxs_ap=srcp_sb[:, k0 * 8:(k0 + nkb) * 8],
                    num_idxs=nkb * 128, num_idxs_reg=cnt_regs[b],
                    elem_size=2 * C2, single_packet=False,
                    queue_num=j % 4)

            def conv_rows(src_sb):
                """src [128n, 256] f32 -> grouped conv rows [128n, 256] dt."""
                yps = pp.tile([128, C2], F32, tag="mmB")
                for g in (0, 1):
                    tp = pp.tile([128, 128], dtm, tag="tps")
                    nc.tensor.transpose(out=tp[:, :],
                                        in_=src_sb[:, g * 128:(g + 1) * 128],
                                        identity=ident[:, :])
                    tsb = wp.tile([128, 128], dtm, tag="tsb")
                    if g == 0:
                        nc.vector.tensor_copy(out=tsb[:, :], in_=tp[:, :])
                    else:
                        nc.scalar.copy(out=tsb[:, :], in_=tp[:, :])
                    nc.tensor.matmul(out=yps[:, g * 128:(g + 1) * 128],
                                     lhsT=tsb[:, :],
                                     rhs=wct_sb[:, g * 128:(g + 1) * 128],
                                     start=True, stop=True)
                ysb = wp.tile([128, C2], dt, tag="ysb")
                nc.scalar.copy(out=ysb[:, :], in_=yps[:, :])
                return ysb

            def spmm_block(b, j):
                """Aggregate one 128-dst block: fire the gather, accumulate
                the sel matmuls in PSUM on top of the merge term."""
                nkb, k0 = nk[b], koff[b]
                direct_gather(b, j)
                gt = gt_tiles.pop(j)
                zps = pp.tile([128, C2], F32, tag="mmA")
                # merge term x_b @ W_merge opens the accumulation group; it
                # only reads resident tiles, so the PE runs it while the
                # gather DMA is still landing.
                nc.tensor.matmul(out=zps[:, :],
                                 lhsT=xT_sb[:, b * 128:(b + 1) * 128],
                                 rhs=wmrg_sb[:, :], start=True, stop=False)
                mi = 0
                for k in range(nkb):
                    for par, off in ((0, 0), (1, C2)):
                        nc.tensor.matmul(
                            out=zps[:, :],
                            lhsT=selm_sb[:, (k0 + k) * 256 + par * 128:
                                         (k0 + k) * 256 + (par + 1) * 128],
                            rhs=gt[:, k, off:off + C2],
                            start=False, stop=(mi == 2 * nkb - 1))
                        mi += 1
                return zps

            # ---------------- phase 1: dense init + y1 rows
            for b in range(cfg.nblk):
                xb = xT_sb[:, b * 128:(b + 1) * 128]
                ps1 = pp.tile([128, C2], F32, tag="mmA")
                nc.tensor.matmul(out=ps1[:, :], lhsT=xb, rhs=wsum_sb[:, :],
                                 start=True, stop=True)
                o1 = wp.tile([128, C2], F32, tag="o1")
                nc.vector.tensor_tensor(out=o1[:, :], in0=ps1[:, :],
                                        in1=b1_sb[:, :], op=add)
                o1r = wp.tile([128, C2], dtm, tag="o1r")
                nc.scalar.activation(out=o1r[:, :], in_=o1[:, :], func=relu)
                ysb = conv_rows(o1r)
                s, jseg = b // cfg.segblk, b % cfg.segblk
                nc.sync.dma_start(
                    out=y_own[0][s][jseg * 128:(jseg + 1) * 128, :],
                    in_=ysb[:, :])
                if jseg == cfg.segblk - 1:
                    nc.gpsimd.collective_compute(
                        "AllGather", mybir.AluOpType.bypass, replica_groups=rg,
                        ins=[y_own[0][s][:, :]],
                        outs=[y_full[0][s * N_CORES * cfg.segrows:
                                        (s + 1) * N_CORES * cfg.segrows, :]])

            # ---------------- phase 2: z1 -> out2 -> y2 rows
            # AG for segment s is issued AG_DELAY blocks after its last
            # producer block, so the CC-busy wait overlaps gather desc-gen
            # instead of stalling it at the gpsimd engine head.
            AG_DELAY = 5

            def emit_ag2(s):
                nc.gpsimd.collective_compute(
                    "AllGather", mybir.AluOpType.bypass, replica_groups=rg,
                    ins=[y_own[1][s][:, :]],
                    outs=[y_full[1][s * N_CORES * cfg.segrows:
                                    (s + 1) * N_CORES * cfg.segrows, :]])

            for b in range(cfg.nblk):
                zps = spmm_block(b, b)
                o2 = wp.tile([128, C2], F32, tag="o1")
                nc.vector.tensor_tensor(out=o2[:, :], in0=zps[:, :],
                                        in1=b2_sb[:, :], op=add)
                o2r = wp.tile([128, C2], dtm, tag="o1r")
                nc.scalar.activation(out=o2r[:, :], in_=o2[:, :], func=relu)
                ysb = conv_rows(o2r)
                s, jseg = b // cfg.segblk, b % cfg.segblk
                nc.sync.dma_start(
                    out=y_own[1][s][jseg * 128:(jseg + 1) * 128, :],
                    in_=ysb[:, :])
                if (b >= cfg.segblk - 1 + AG_DELAY
                        and (b - AG_DELAY) % cfg.segblk == cfg.segblk - 1):
                    emit_ag2((b - AG_DELAY) // cfg.segblk)
            for s in range((cfg.nblk - AG_DELAY) // cfg.segblk, cfg.segs):
                emit_ag2(s)

            # ---------------- phase 3: z2 -> out3 -> pooled output rows
            for b in range(cfg.nblk):
                zps = spmm_block(b, cfg.nblk + b)
                o3 = wp.tile([128, C2], F32, tag="o1")
                nc.vector.tensor_tensor(out=o3[:, :], in0=zps[:, :],
                                        in1=b2_sb[:, :], op=add)
                o3r = wp.tile([128, C2], F32, tag="o3r")
                nc.scalar.activation(out=o3r[:, :], in_=o3[:, :], func=relu)
                res = wp.tile([128, 128], F32, tag="res")
                for g in (0, 1):
                    ev = o3r[:, g * 128:(g + 1) * 128:2]
                    od = o3r[:, g * 128 + 1:(g + 1) * 128:2]
                    pm = wp.tile([128, 64], F32, tag="pm")
                    nc.vector.tensor_tensor(out=pm[:, :], in0=ev, in1=od,
                                            op=add)
                    nc.scalar.activation(
                        out=res[:, g * 64:(g + 1) * 64], in_=pm[:, :],
                        func=mybir.ActivationFunctionType.Copy, scale=0.5)
                nc.sync.dma_start(out=out_d[b * 128:(b + 1) * 128, :],
                                  in_=res[:, :])

    nc.finalize()
    return nc


# ------------------------------------------------------------------ runner

_PROG_CACHE = {}


def get_program(cfg, nk):
    key = (cfg.n, cfg.nblk, cfg.segs, str(cfg.dt_tab), nk)
    if key not in _PROG_CACHE:
        _PROG_CACHE[key] = build_program(cfg, nk)
    return _PROG_CACHE[key]


def kernel(node_tensor, edge_index, W_pre, b_pre, W_merge, b_merge,
           W_conv, b_conv, _cfg=None, **_run_kwargs):
    cfg = _cfg or full_cfg()
    per_core, nk = host_prep(node_tensor, edge_index, W_pre, b_pre, W_merge,
                             b_merge, W_conv, b_conv, cfg)
    nc = get_program(cfg, nk)
    res = run_bass_kernel_spmd(nc, per_core, core_ids=list(range(N_CORES)),
                               **_run_kwargs)
    outs = np.concatenate([res.results[c]["out"] for c in range(N_CORES)],
                          axis=0)
    full = outs[:cfg.n].astype(np.float32)[:, :, None]
    if _run_kwargs:
        return full, res
    return full
